# revision 11
# baseline (speedup 1.0000x reference)
"""Trainium2 Bass kernel for nn_CrossAttention (self-attention, B=2 N=4096 D=512 H=8 DH=64).

Sharding: 8 cores = 2 batches x 4 query-row slices (1024 rows each). Every core
holds the full 4096-token batch slice for K/V (recomputed per core -- cheap) and
computes attention + output projection for its 1024 query rows entirely on-chip
(flash-attention style: the [4096, 4096] score matrix never touches HBM).
Host-side work is only input slicing and output concatenation.
"""

import os
import sys
from contextlib import ExitStack

import numpy as np

for _p in ("/opt/trn_rl_repo", "/root/.axon_site/_ro/trn_rl_repo"):
    if os.path.isdir(_p) and _p not in sys.path:
        sys.path.insert(0, _p)

import concourse.bass as bass
from concourse import bacc
import concourse.mybir as mybir
import concourse.tile as tile
from concourse.bass_utils import run_bass_kernel_spmd
from concourse.masks import make_identity

F32 = mybir.dt.float32
EXP = mybir.ActivationFunctionType.Exp

# Problem dims (hardcoded per spec)
B, N, D = 2, 4096, 512
H, DH = 8, 64
SCALE = DH ** -0.5
NCORES = 8
CORES_PER_B = NCORES // B      # 4
NQ = N // CORES_PER_B          # 1024 query rows per core

# matmul operand dtype: float32r = single-pass (4x faster) PE mode, fp32 storage.
# Toggle with ATTN_FP32=1 for the exact-fp32 (2-pass, slower) variant.
MM_DT = F32 if os.environ.get("ATTN_FP32") == "1" else mybir.dt.float32r


def build_nc(mm_dt=MM_DT, n=N, nq=NQ, d=D, h=H, dh=DH):
    """Build the single-core Bass program (same program runs SPMD on all 8 cores)."""
    assert d == 512 and h == 8 and dh == 64
    assert n % 512 == 0 and nq % 128 == 0
    qts = 512 if nq % 512 == 0 else nq      # query-tile size
    assert qts <= 512 and nq % qts == 0
    njc = n // 128                          # 128-token key chunks
    nch = n // 512                          # 512-row x chunks
    nqch = nq // 512 if nq % 512 == 0 else 1

    nc = bacc.Bacc(None, target_bir_lowering=False)
    x_d = nc.dram_tensor("x", [n, d], F32, kind="ExternalInput")
    xq_d = nc.dram_tensor("xq", [nq, d], F32, kind="ExternalInput")
    wq_d = nc.dram_tensor("wq", [d, d], F32, kind="ExternalInput")
    wk_d = nc.dram_tensor("wk", [d, d], F32, kind="ExternalInput")
    wv_d = nc.dram_tensor("wv", [d, d], F32, kind="ExternalInput")
    wo_d = nc.dram_tensor("wo", [d, d], F32, kind="ExternalInput")
    bo_d = nc.dram_tensor("bo", [d], F32, kind="ExternalInput")
    out_d = nc.dram_tensor("out", [nq, d], F32, kind="ExternalOutput")


    with tile.TileContext(nc) as tc, ExitStack() as ctx:
        persist = ctx.enter_context(tc.tile_pool(name="persist", bufs=1))

        # Persistent SBUF state
        kT = [persist.tile([128, n], mm_dt, tag=f"kT{i}", name=f"kT{i}") for i in range(4)]   # [hd-pair, tokens]
        vsb = persist.tile([128, njc, 8 * 65], mm_dt, tag="vsb")  # per j-chunk: 8x(64 v cols + ones)
        qT = [persist.tile([128, nq], mm_dt, tag=f"qT{i}", name=f"qT{i}") for i in range(4)]
        wo_sb = persist.tile([128, 4, 512], mm_dt, tag="wo")
        bo_bc = persist.tile([128, 512], F32, tag="bo_bc")
        ident = persist.tile([128, 128], F32, tag="ident")

        make_identity(nc, ident)
        # ones columns of v_aug (col 64 of each head block), set once
        # (memset can't write f32r -- copy from an fp32 ones tile, DVE rounds)
        ones_f32 = persist.tile([128, 1], F32, tag="ones_f32")
        nc.vector.memset(ones_f32, 1.0)
        vones = vsb[:].rearrange("p j (h c) -> p j h c", c=65)[:, :, :, 64:65]
        nc.vector.tensor_copy(vones, ones_f32[:, 0:1].broadcast_to((128, njc, 8, 1)))
        # bias broadcast [512] -> [128, 512] via 0-stride DMA
        bo_t = bo_d.tensor if hasattr(bo_d, "tensor") else bo_d
        bo_ap = bass.AP(tensor=bo_t, offset=0, ap=[[0, 128], [1, 512]])
        nc.gpsimd.dma_start(out=bo_bc, in_=bo_ap)
        # Wo: DMA to fp32 staging chunks, DVE copy (rounds to mm_dt)
        with tc.tile_pool(name="wost", bufs=2) as wostp:
            for hc in range(4):
                wo_st = wostp.tile([128, 512], F32, tag="wo_st")
                nc.gpsimd.dma_start(out=wo_st, in_=wo_d[hc * 128:(hc + 1) * 128, :])
                nc.vector.tensor_copy(wo_sb[:, hc, :], wo_st)

        def transpose_chunk(xdram, row0, xpool, trpool, xtpool):
            """Load 512 rows of x and return 4 transposed [128 d, 512 n] tiles."""
            xts = [xtpool.tile([128, 512], mm_dt, tag="xt", name="xt") for _ in range(4)]
            for s in range(4):
                xin = xpool.tile([128, 512], F32, tag="xin")
                nc.gpsimd.dma_start(out=xin, in_=xdram[row0 + s * 128: row0 + (s + 1) * 128, :])
                for dc in range(4):
                    ptr = trpool.tile([128, 128], F32, tag="tr")
                    nc.tensor.transpose(ptr, xin[:, dc * 128:(dc + 1) * 128], ident)
                    nc.vector.tensor_copy(xts[dc][:, s * 128:(s + 1) * 128], ptr)
            return xts

        # ---- Phase A: qT projection (query rows) ----
        with tc.tile_pool(name="wqp", bufs=1) as wqp, \
             tc.tile_pool(name="xa", bufs=3) as xpool, \
             tc.tile_pool(name="tra", bufs=2, space="PSUM") as trpool, \
             tc.tile_pool(name="xta", bufs=4) as xtpool, \
             tc.tile_pool(name="pja", bufs=2, space="PSUM") as pjpool:
            wq_sb = wqp.tile([128, 4, 512], mm_dt, tag="wq")
            for dc in range(4):
                wq_st = xpool.tile([128, 512], F32, tag="xin")
                nc.gpsimd.dma_start(out=wq_st, in_=wq_d[dc * 128:(dc + 1) * 128, :])
                nc.vector.tensor_copy(wq_sb[:, dc, :], wq_st)
            for qch in range(max(1, nq // 512)):
                row0 = qch * 512
                rows = min(512, nq - row0)
                xts = transpose_chunk(xq_d, row0, xpool, trpool, xtpool) if rows == 512 else None
                if xts is None:
                    # small-config path (nq < 512): transpose rows we have
                    xts = [xtpool.tile([128, 512], mm_dt, tag="xt", name="xt") for _ in range(4)]
                    for s in range(rows // 128):
                        xin = xpool.tile([128, 512], F32, tag="xin")
                        nc.gpsimd.dma_start(out=xin, in_=xq_d[row0 + s * 128: row0 + (s + 1) * 128, :])
                        for dc in range(4):
                            ptr = trpool.tile([128, 128], F32, tag="tr")
                            nc.tensor.transpose(ptr, xin[:, dc * 128:(dc + 1) * 128], ident)
                            nc.vector.tensor_copy(xts[dc][:, s * 128:(s + 1) * 128], ptr)
                for hc in range(4):
                    pq = pjpool.tile([128, 512], F32, tag="pj")
                    for dc in range(4):
                        nc.tensor.matmul(pq[:, :rows], (wq_sb[:, dc, hc * 128:(hc + 1) * 128]),
                                         (xts[dc][:, :rows]), start=(dc == 0), stop=(dc == 3))
                    nc.vector.tensor_copy(qT[hc][:, row0: row0 + rows], pq[:, :rows])

        # ---- Phase B: kT and v projections (all tokens) ----
        with tc.tile_pool(name="wkvp", bufs=1) as wkvp, \
             tc.tile_pool(name="xb", bufs=3) as xpool, \
             tc.tile_pool(name="trb", bufs=2, space="PSUM") as trpool, \
             tc.tile_pool(name="xtb", bufs=4) as xtpool, \
             tc.tile_pool(name="pjb", bufs=2, space="PSUM") as pjpool:
            wk_sb = wkvp.tile([128, 4, 512], mm_dt, tag="wk")
            wv_sb = wkvp.tile([128, 4, 512], mm_dt, tag="wv")
            for dc in range(4):
                wk_st = xpool.tile([128, 512], F32, tag="xin")
                nc.gpsimd.dma_start(out=wk_st, in_=wk_d[dc * 128:(dc + 1) * 128, :])
                nc.vector.tensor_copy(wk_sb[:, dc, :], wk_st)
                wv_st = xpool.tile([128, 512], F32, tag="xin")
                nc.gpsimd.dma_start(out=wv_st, in_=wv_d[dc * 128:(dc + 1) * 128, :])
                nc.vector.tensor_copy(wv_sb[:, dc, :], wv_st)
            for ch in range(nch):
                xts = transpose_chunk(x_d, ch * 512, xpool, trpool, xtpool)
                # kT[:, chunk]: lhsT = Wk d-chunk cols, rhs = xT
                for hc in range(4):
                    pk = pjpool.tile([128, 512], F32, tag="pj")
                    for dc in range(4):
                        nc.tensor.matmul(pk, (wk_sb[:, dc, hc * 128:(hc + 1) * 128]),
                                         (xts[dc]), start=(dc == 0), stop=(dc == 3))
                    nc.vector.tensor_copy(kT[hc][:, ch * 512:(ch + 1) * 512], pk)
                # v natural [n, hd] for 4 j-chunks of this x chunk
                for s in range(4):
                    pv = pjpool.tile([128, 512], F32, tag="pj")
                    for dc in range(4):
                        nc.tensor.matmul(pv, (xts[dc][:, s * 128:(s + 1) * 128]),
                                         (wv_sb[:, dc, :]), start=(dc == 0), stop=(dc == 3))
                    vdst = vsb[:, ch * 4 + s, :].rearrange("p (h c) -> p h c", c=65)[:, :, 0:64]
                    vsrc = pv[:].rearrange("p (h c) -> p h c", c=64)
                    nc.vector.tensor_copy(vdst, vsrc)

        # ---- Phase C: attention + output projection ----
        with tc.tile_pool(name="stp", bufs=2, space="PSUM") as stpool, \
             tc.tile_pool(name="otp", bufs=2, space="PSUM") as otpool, \
             tc.tile_pool(name="pop", bufs=2, space="PSUM") as popool, \
             tc.tile_pool(name="exp", bufs=2) as expool, \
             tc.tile_pool(name="ocp", bufs=4) as ocpool, \
             tc.tile_pool(name="serp", bufs=2) as serpool, \
             tc.tile_pool(name="bcp", bufs=2) as bcpool, \
             tc.tile_pool(name="bncp", bufs=2, space="DRAM") as bncpool, \
             tc.tile_pool(name="outp", bufs=3) as outpool:
            vre = vsb[:].rearrange("p j (h c) -> p j h c", c=65)
            for qt in range(nq // qts):
                ocat = []
                for hp in range(4):
                    oTa = otpool.tile([65, qts], F32, tag="oT")
                    oTb = otpool.tile([65, qts], F32, tag="oT")
                    for j in range(njc):
                        st = stpool.tile([128, 2 * qts], F32, tag="st")
                        nc.tensor.matmul(st[:, 0:qts],
                                         (kT[hp][0:64, j * 128:(j + 1) * 128]),
                                         (qT[hp][0:64, qt * qts:(qt + 1) * qts]),
                                         start=True, stop=True)
                        nc.tensor.matmul(st[:, qts:2 * qts],
                                         (kT[hp][64:128, j * 128:(j + 1) * 128]),
                                         (qT[hp][64:128, qt * qts:(qt + 1) * qts]),
                                         start=True, stop=True)
                        ex = expool.tile([128, 2 * qts], mm_dt, tag="ex")
                        nc.scalar.activation(ex, st, EXP, scale=SCALE)
                        nc.tensor.matmul(oTa, (vre[:, j, 2 * hp, :]), (ex[:, 0:qts]),
                                         start=(j == 0), stop=(j == njc - 1))
                        nc.tensor.matmul(oTb, (vre[:, j, 2 * hp + 1, :]), (ex[:, qts:2 * qts]),
                                         start=(j == 0), stop=(j == njc - 1))
                    # epilogue: recip of sumexp (row 64), broadcast, normalize
                    ser = serpool.tile([1, 2 * qts], F32, tag="ser")
                    nc.vector.reciprocal(ser[0:1, 0:qts], oTa[64:65, :])
                    nc.vector.reciprocal(ser[0:1, qts:2 * qts], oTb[64:65, :])
                    dbnc = bncpool.tile([1, 2 * qts], F32, tag="dbnc")
                    nc.gpsimd.dma_start(out=dbnc[:], in_=ser[:])
                    bc = bcpool.tile([128, qts], F32, tag="bc")
                    nc.gpsimd.dma_start(out=bc[0:64, :],
                                      in_=dbnc[0:1, 0:qts].broadcast_to((64, qts)))
                    nc.gpsimd.dma_start(out=bc[64:128, :],
                                      in_=dbnc[0:1, qts:2 * qts].broadcast_to((64, qts)))
                    oc = ocpool.tile([128, qts], mm_dt, tag="ocat")
                    nc.vector.tensor_mul(oc[0:64, :], oTa[0:64, :], bc[0:64, :])
                    nc.vector.tensor_mul(oc[64:128, :], oTb[0:64, :], bc[64:128, :])
                    ocat.append(oc)
                # output projection: out[i, :] = sum_hp ocatT[hp].T @ Wo[hp] + bo
                for it in range(qts // 128):
                    po = popool.tile([128, 512], F32, tag="po")
                    for hp in range(4):
                        nc.tensor.matmul(po, (ocat[hp][:, it * 128:(it + 1) * 128]),
                                         (wo_sb[:, hp, :]), start=(hp == 0), stop=(hp == 3))
                    ot = outpool.tile([128, 512], F32, tag="ot")
                    nc.vector.tensor_add(ot, po, bo_bc)
                    nc.gpsimd.dma_start(out=out_d[qt * qts + it * 128: qt * qts + (it + 1) * 128, :],
                                      in_=ot)
    nc.finalize()
    return nc


_NC_CACHE = {}


def _get_nc(key="main"):
    if key not in _NC_CACHE:
        _NC_CACHE[key] = build_nc()
    return _NC_CACHE[key]


def _make_in_maps(inputs):
    x = np.ascontiguousarray(np.asarray(inputs["x"], dtype=np.float32))
    wq = np.ascontiguousarray(np.asarray(inputs["Wq"], dtype=np.float32))
    wk = np.ascontiguousarray(np.asarray(inputs["Wk"], dtype=np.float32))
    wv = np.ascontiguousarray(np.asarray(inputs["Wv"], dtype=np.float32))
    wo = np.ascontiguousarray(np.asarray(inputs["Wo"], dtype=np.float32))
    bo = np.ascontiguousarray(np.asarray(inputs["bo"], dtype=np.float32))
    in_maps = []
    for c in range(NCORES):
        b = c // CORES_PER_B
        r0 = (c % CORES_PER_B) * NQ
        in_maps.append({
            "x": np.ascontiguousarray(x[b]),
            "xq": np.ascontiguousarray(x[b, r0:r0 + NQ]),
            "wq": wq, "wk": wk, "wv": wv, "wo": wo, "bo": bo,
        })
    return in_maps


def _assemble(results):
    out = np.empty((B, N, D), dtype=np.float32)
    for c in range(NCORES):
        b = c // CORES_PER_B
        r0 = (c % CORES_PER_B) * NQ
        out[b, r0:r0 + NQ] = results[c]["out"]
    return out


def kernel(**inputs) -> np.ndarray:
    nc = _get_nc()
    res = run_bass_kernel_spmd(nc, _make_in_maps(inputs), core_ids=list(range(NCORES)))
    return _assemble(res.results)


def kernel_traced(**inputs):
    """Returns (output, exec_time_ns_or_None). NTFF tracing when available."""
    nc = _get_nc()
    try:
        res = run_bass_kernel_spmd(nc, _make_in_maps(inputs), core_ids=list(range(NCORES)),
                                   trace=True)
    except (ModuleNotFoundError, ImportError):
        res = run_bass_kernel_spmd(nc, _make_in_maps(inputs), core_ids=list(range(NCORES)))
    return _assemble(res.results), res.exec_time_ns, res
